# revision 2
# baseline (speedup 1.0000x reference)
"""Trainium2 Bass kernel for nn_Convert2Dto3DWithPadding.

Problem: x [204800, 128] f32 ragged atom features + sorted batch_ids [204800]
-> (result [4096, 128, 128] f32 padded per-graph tensor, mask [4096, 128] bool).

Strategy (data-parallel over graphs, per the sharding hint):
  - Host: split the 4096 graphs into 8 contiguous ranges of 512 graphs; each
    core owns its graphs' atoms (a contiguous slice of x, since batch_ids is
    sorted). Host computes, per core, the destination row of every atom
    (dest = local_graph*128 + within-graph position) and the list of padding
    rows, and ships them as small int32 index tensors (~0.4% of data bytes).
  - Device (SPMD, one program for all 8 cores): dense-load the local x slice
    into SBUF with large DMAs, then SWDGE indirect-scatter rows to their
    padded positions in DRAM (512 B/row descriptors run at DMA line rate).
    Padding rows are zero-filled by scattering from a zeroed SBUF buffer,
    mostly as 4-row (2 KiB) chunks. Every output row is written exactly once:
    HBM traffic per core = 13.2 MB read + 33.6 MB write, the memory roofline.
    The mask is computed on-chip (iota < counts) and stored directly.

Self-contained: geometry is hardcoded; inputs arrive as full numpy arrays.
"""

import numpy as np

TOTAL_ATOMS = 204800
NUM_GRAPHS = 4096
NF = 128          # features per atom (row = 512 B)
MAXA = 128        # padded atoms per graph
NCORES = 8
GPC = NUM_GRAPHS // NCORES      # graphs per core = 512
RPC = GPC * MAXA                # output rows per core = 65536

NCHUNK = 8                      # data pipeline chunks
W = 27                          # x columns per partition per chunk
NL = 128 * W * NCHUNK           # local x capacity = 27648 rows (actual ~25.7k)
Z4_CALLS = 5                    # 4-row zero-scatter calls
Z4_COLS = 16                    # each call: 128*16 = 2048 chunk indices
Z1_COLS = 12                    # 1-row zero-scatter: 1536 slots (>= max 3*512)
ZBUF_COLS = 64                  # zero source: 128*64 = 8192 rows (2048 idx * 4)
SENT = np.int32(1 << 20)        # out-of-bounds sentinel; dropped by bounds check

_PROG = None


def _build_program():
    import concourse.bacc as bacc
    import concourse.mybir as mybir
    import concourse.tile as tile
    from concourse.bass import IndirectOffsetOnAxis

    f32, i32, u8 = mybir.dt.float32, mybir.dt.int32, mybir.dt.uint8

    nc = bacc.Bacc("TRN2", debug=False, num_devices=NCORES, enable_asserts=False)

    xin = nc.dram_tensor("xin", [NL, NF], f32, kind="ExternalInput")
    didx = nc.dram_tensor("didx", [NCHUNK, 128, W], i32, kind="ExternalInput")
    pidx4 = nc.dram_tensor("pidx4", [Z4_CALLS, 128, Z4_COLS], i32, kind="ExternalInput")
    pidx1 = nc.dram_tensor("pidx1", [128, Z1_COLS], i32, kind="ExternalInput")
    cnts = nc.dram_tensor("cnts", [128, GPC // 128], f32, kind="ExternalInput")
    out = nc.dram_tensor("out", [RPC, NF], f32, kind="ExternalOutput")
    mout = nc.dram_tensor("mask", [GPC, MAXA], u8, kind="ExternalOutput")

    with tile.TileContext(nc) as tc:
        with (
            tc.tile_pool(name="xp", bufs=4) as xp,
            tc.tile_pool(name="ip", bufs=4) as ip,
            tc.tile_pool(name="zp", bufs=1) as zp,
            tc.tile_pool(name="zi", bufs=2) as zi,
            tc.tile_pool(name="mp", bufs=1) as mp,
        ):
            # mask = iota(128) < counts, per graph (graph p*4+b at [p, b])
            nb = GPC // 128
            cnt = mp.tile([128, nb], f32)
            nc.sync.dma_start(out=cnt[:], in_=cnts.ap()[:, :])
            io = mp.tile([128, MAXA], f32)
            nc.gpsimd.iota(io[:], pattern=[[1, MAXA]], base=0, channel_multiplier=0,
                           allow_small_or_imprecise_dtypes=True)
            msk = mp.tile([128, nb, MAXA], u8)
            for b in range(nb):
                nc.vector.tensor_scalar(
                    out=msk[:, b, :], in0=io[:], scalar1=cnt[:, b:b + 1],
                    scalar2=None, op0=mybir.AluOpType.is_lt,
                )
            nc.sync.dma_start(out=mout.ap()[:, :], in_=msk[:])

            # data: dense-load x chunk, indirect-scatter rows to padded slots
            for k in range(NCHUNK):
                rows = 128 * W
                xt = xp.tile([128, W, NF], f32)
                nc.sync.dma_start(out=xt[:], in_=xin.ap()[k * rows:(k + 1) * rows, :])
                ix = ip.tile([128, W], i32)
                nc.sync.dma_start(out=ix[:], in_=didx.ap()[k])
                nc.gpsimd.indirect_dma_start(
                    out=out.ap()[:, :],
                    out_offset=IndirectOffsetOnAxis(ap=ix[:], axis=0),
                    in_=xt[:],
                    in_offset=None,
                    bounds_check=RPC - 1,
                    oob_is_err=False,
                )

            # zero-fill padding rows from a zeroed SBUF buffer
            zbuf = zp.tile([128, ZBUF_COLS, NF], f32)
            nc.vector.memset(zbuf[:], 0.0)
            for z in range(Z4_CALLS):  # 4 rows (2 KiB) per index
                pz = zi.tile([128, Z4_COLS], i32)
                nc.sync.dma_start(out=pz[:], in_=pidx4.ap()[z])
                nc.gpsimd.indirect_dma_start(
                    out=out.ap()[:, :],
                    out_offset=IndirectOffsetOnAxis(ap=pz[:], axis=0),
                    in_=zbuf[:],
                    in_offset=None,
                    bounds_check=RPC - 1,
                    oob_is_err=False,
                )
            p1 = zi.tile([128, Z1_COLS], i32)  # remainder: 1 row per index
            nc.sync.dma_start(out=p1[:], in_=pidx1.ap()[:, :])
            nc.gpsimd.indirect_dma_start(
                out=out.ap()[:, :],
                out_offset=IndirectOffsetOnAxis(ap=p1[:], axis=0),
                in_=zbuf[:, 0:Z1_COLS, :],
                in_offset=None,
                bounds_check=RPC - 1,
                oob_is_err=False,
            )

    nc.compile()
    return nc


def _get_program():
    global _PROG
    if _PROG is None:
        _PROG = _build_program()
    return _PROG


def _prep_inputs(x, batch_ids):
    """Host-side shard + index prep. Returns per-core input maps."""
    x = np.ascontiguousarray(np.asarray(x), dtype=np.float32)
    bids = np.asarray(batch_ids).astype(np.int64)
    assert x.shape == (TOTAL_ATOMS, NF), x.shape
    assert bids.shape == (TOTAL_ATOMS,)

    counts = np.bincount(bids, minlength=NUM_GRAPHS)[:NUM_GRAPHS]
    starts = np.cumsum(counts) - counts
    pos = np.arange(TOTAL_ATOMS, dtype=np.int64) - starts[bids]

    in_maps = []
    for c in range(NCORES):
        g0, g1 = c * GPC, (c + 1) * GPC
        a0 = int(starts[g0])
        a1 = int(starts[g1]) if g1 < NUM_GRAPHS else TOTAL_ATOMS
        posl = pos[a0:a1]
        keep = posl < MAXA  # reference scatter uses mode="drop"
        xs = x[a0:a1][keep]
        n_loc = xs.shape[0]
        assert n_loc <= NL, (c, n_loc, NL)
        destl = ((bids[a0:a1][keep] - g0) * MAXA + posl[keep]).astype(np.int32)

        xin = np.zeros((NL, NF), np.float32)
        xin[:n_loc] = xs
        didx = np.full(NL, SENT, np.int32)
        didx[:n_loc] = destl
        didx = didx.reshape(NCHUNK, 128, W)

        # padding rows [count, 128) per graph, as 4-row chunks + 1-row remainder
        ce = np.minimum(counts[g0:g1], MAXA).astype(np.int64)
        base = np.arange(GPC, dtype=np.int64) * MAXA
        n4 = (MAXA - ce) // 4
        r1 = (MAXA - ce) % 4
        tot4 = int(n4.sum())
        z4 = (np.repeat(base + ce, n4)
              + (np.arange(tot4) - np.repeat(np.cumsum(n4) - n4, n4)) * 4)
        tot1 = int(r1.sum())
        z1 = (np.repeat(base + MAXA - r1, r1)
              + (np.arange(tot1) - np.repeat(np.cumsum(r1) - r1, r1)))
        assert z4.size <= Z4_CALLS * 128 * Z4_COLS, z4.size
        assert z1.size <= 128 * Z1_COLS, z1.size
        pidx4 = np.full(Z4_CALLS * 128 * Z4_COLS, SENT, np.int32)
        pidx4[:z4.size] = z4
        pidx1 = np.full(128 * Z1_COLS, SENT, np.int32)
        pidx1[:z1.size] = z1

        in_maps.append({
            "xin": xin,
            "didx": didx,
            "pidx4": pidx4.reshape(Z4_CALLS, 128, Z4_COLS),
            "pidx1": pidx1.reshape(128, Z1_COLS),
            "cnts": counts[g0:g1].astype(np.float32).reshape(128, GPC // 128),
        })
    return in_maps


def _assemble(results):
    res = np.concatenate([r["out"] for r in results], axis=0)
    res = res.reshape(NUM_GRAPHS, MAXA, NF)
    mask = np.concatenate([r["mask"] for r in results], axis=0) != 0
    return res, mask


def kernel(x, batch_ids, num_graphs, max_num_atoms):
    assert int(num_graphs) == NUM_GRAPHS and int(max_num_atoms) == MAXA
    from concourse.bass_utils import run_bass_kernel_spmd

    nc = _get_program()
    in_maps = _prep_inputs(x, batch_ids)
    res = run_bass_kernel_spmd(nc, in_maps, core_ids=list(range(NCORES)))
    return _assemble(res.results)


# revision 3
# speedup vs baseline: 11.7759x; 11.7759x over previous
"""Trainium2 Bass kernel for nn_Convert2Dto3DWithPadding.

Problem: x [204800, 128] f32 ragged atom features + sorted batch_ids [204800]
-> (result [4096, 128, 128] f32 padded per-graph tensor, mask [4096, 128] bool).

Strategy (data-parallel over graphs, per the sharding hint):
  - Host: split the 4096 graphs into 8 contiguous ranges of 512; each core owns
    its graphs' atoms (a contiguous slice of x, since batch_ids is sorted).
    Because ids are sorted, the scatter is pure contiguous block copies: graph
    g's count_g atoms go to output rows [g*128, g*128+count_g), the rest of the
    block is zeros. The host decomposes every graph's data run and padding run
    into blocks of {32,16,8,4,2,1} rows, reorders x into per-class regions
    (one block per SBUF partition), and ships per-block destination rows as an
    int32 index tensor.
  - Device (SPMD, one program for all 8 cores): per class, dense-load 128
    blocks per call into SBUF and SWDGE indirect-scatter them with a [128,1]
    index AP (one index per partition - the HW-supported form; validated by
    probes). Padding blocks scatter from a zeroed SBUF buffer. Unused call
    slots are aimed at scratch rows past the real output, which the host
    discards. Every real output row is written exactly once: HBM traffic per
    core is ~13.2 MB read + ~33.6 MB write, the memory-bandwidth roofline.
    The mask is computed on-chip (iota < counts) and stored directly.

Self-contained: geometry is hardcoded; inputs arrive as full numpy arrays.
"""

import numpy as np

TOTAL_ATOMS = 204800
NUM_GRAPHS = 4096
NF = 128          # features per atom (row = 512 B)
MAXA = 128        # padded atoms per graph
NCORES = 8
GPC = NUM_GRAPHS // NCORES      # graphs per core = 512
RPC = GPC * MAXA                # real output rows per core = 65536
NDUMP = 128                     # scratch rows for unused scatter slots

SIZES = [32, 16, 8, 4, 2, 1]    # block heights (rows)
CAP_DATA = [576, 384, 320, 320, 320, 320]    # per-class block capacity
CAP_ZERO = [1088, 320, 320, 320, 320, 320]

def _calls(cap):
    out = []
    done = 0
    while done < cap:
        out.append(min(128, cap - done))
        done += 128
    return out

CALLS_DATA = [_calls(c) for c in CAP_DATA]   # per class: partitions per call
CALLS_ZERO = [_calls(c) for c in CAP_ZERO]
T_ZERO = sum(len(c) for c in CALLS_ZERO)
T_DATA = sum(len(c) for c in CALLS_DATA)
T_ALL = T_ZERO + T_DATA

_PROG = None


def _build_program():
    import concourse.bacc as bacc
    import concourse.mybir as mybir
    import concourse.tile as tile
    from concourse.bass import IndirectOffsetOnAxis

    f32, i32, u8 = mybir.dt.float32, mybir.dt.int32, mybir.dt.uint8

    nc = bacc.Bacc("TRN2", debug=False, num_devices=NCORES, enable_asserts=False)

    xcs = [
        nc.dram_tensor(f"xc{s}", [CAP_DATA[ci], s * NF], f32, kind="ExternalInput")
        for ci, s in enumerate(SIZES)
    ]
    idx_all = nc.dram_tensor("idx", [128, T_ALL], i32, kind="ExternalInput")
    cnts = nc.dram_tensor("cnts", [128, GPC // 128], f32, kind="ExternalInput")
    out = nc.dram_tensor("out", [RPC + NDUMP, NF], f32, kind="ExternalOutput")
    mout = nc.dram_tensor("mask", [GPC, MAXA], u8, kind="ExternalOutput")

    with tile.TileContext(nc) as tc:
        with (
            tc.tile_pool(name="xp", bufs=2) as xp,
            tc.tile_pool(name="zp", bufs=1) as zp,
            tc.tile_pool(name="mp", bufs=1) as mp,
        ):
            ix = mp.tile([128, T_ALL], i32)
            nc.sync.dma_start(out=ix[:], in_=idx_all.ap()[:, :])

            # mask = iota(128) < counts, graph p*4+b at [p, b]
            nb = GPC // 128
            cnt = mp.tile([128, nb], f32)
            nc.sync.dma_start(out=cnt[:], in_=cnts.ap()[:, :])
            io = mp.tile([128, MAXA], f32)
            nc.gpsimd.iota(io[:], pattern=[[1, MAXA]], base=0, channel_multiplier=0,
                           allow_small_or_imprecise_dtypes=True)
            msk = mp.tile([128, nb, MAXA], u8)
            for b in range(nb):
                nc.vector.tensor_scalar(
                    out=msk[:, b, :], in0=io[:], scalar1=cnt[:, b:b + 1],
                    scalar2=None, op0=mybir.AluOpType.is_lt,
                )
            nc.sync.dma_start(out=mout.ap()[:, :], in_=msk[:])

            def scatter(up, col, src):
                nc.gpsimd.indirect_dma_start(
                    out=out.ap()[:, :],
                    out_offset=IndirectOffsetOnAxis(ap=ix[0:up, col:col + 1], axis=0),
                    in_=src,
                    in_offset=None,
                )

            # zero-fill padding blocks from one zeroed SBUF buffer
            zbuf = zp.tile([128, SIZES[0] * NF], f32)
            nc.vector.memset(zbuf[:], 0.0)
            t = 0
            for ci, s in enumerate(SIZES):
                for up in CALLS_ZERO[ci]:
                    scatter(up, t, zbuf[0:up, 0:s * NF])
                    t += 1

            # data blocks: load 128 blocks (one per partition), then scatter
            for ci, s in enumerate(SIZES):
                done = 0
                for up in CALLS_DATA[ci]:
                    xt = xp.tile([128, s * NF], f32, tag=f"x{s}")
                    nc.sync.dma_start(
                        out=xt[0:up, :], in_=xcs[ci].ap()[done:done + up, :])
                    scatter(up, t, xt[0:up, :])
                    t += 1
                    done += up
            assert t == T_ALL

    nc.compile()
    return nc


def _get_program():
    global _PROG
    if _PROG is None:
        _PROG = _build_program()
    return _PROG


def _decompose(vals):
    """Greedy block decomposition of per-graph run lengths.

    Returns per class: (graph_index[], row_start_within_run[]).
    """
    rem = vals.astype(np.int64).copy()
    koff = np.zeros_like(rem)
    per_class = []
    for s in SIZES:
        n = rem // s
        tot = int(n.sum())
        gidx = np.repeat(np.arange(vals.size), n)
        within = np.arange(tot) - np.repeat(np.cumsum(n) - n, n)
        kstart = koff[gidx] + within * s
        per_class.append((gidx, kstart))
        koff = koff + n * s
        rem = rem - n * s
    return per_class


def _prep_inputs(x, batch_ids):
    """Host-side shard + block-layout prep. Returns per-core input maps."""
    x = np.ascontiguousarray(np.asarray(x), dtype=np.float32)
    bids = np.asarray(batch_ids).astype(np.int64)
    assert x.shape == (TOTAL_ATOMS, NF), x.shape
    assert bids.shape == (TOTAL_ATOMS,)

    counts = np.bincount(bids, minlength=NUM_GRAPHS)[:NUM_GRAPHS]
    starts = np.cumsum(counts) - counts

    in_maps = []
    for c in range(NCORES):
        g0 = c * GPC
        ce = np.minimum(counts[g0:g0 + GPC], MAXA)
        xstart = starts[g0:g0 + GPC]

        idx_cols = np.empty((T_ALL, 128), np.int32)
        t = 0

        # zero blocks: rows [ce, 128) of each graph
        zclasses = _decompose(MAXA - ce)
        for ci, s in enumerate(SIZES):
            gidx, kstart = zclasses[ci]
            dest = (gidx * MAXA + ce[gidx] + kstart).astype(np.int32)
            nb = dest.size
            assert nb <= CAP_ZERO[ci], (c, SIZES[ci], nb)
            done = 0
            for up in CALLS_ZERO[ci]:
                col = np.full(128, RPC + t, np.int32)
                use = max(0, min(up, nb - done))
                col[:use] = dest[done:done + use]
                idx_cols[t] = col
                t += 1
                done += up

        # data blocks: rows [0, ce) of each graph; source rows from x
        dclasses = _decompose(ce)
        xcs = {}
        for ci, s in enumerate(SIZES):
            gidx, kstart = dclasses[ci]
            dest = (gidx * MAXA + kstart).astype(np.int32)
            src = xstart[gidx] + kstart
            nb = dest.size
            assert nb <= CAP_DATA[ci], (c, SIZES[ci], nb)
            xc = np.zeros((CAP_DATA[ci], s * NF), np.float32)
            if nb:
                rows = (src[:, None] + np.arange(s)[None, :]).ravel()
                xc[:nb] = x[rows].reshape(nb, s * NF)
            xcs[f"xc{s}"] = xc
            done = 0
            for up in CALLS_DATA[ci]:
                col = np.full(128, RPC + t, np.int32)
                use = max(0, min(up, nb - done))
                col[:use] = dest[done:done + use]
                idx_cols[t] = col
                t += 1
                done += up
        assert t == T_ALL

        in_maps.append({
            **xcs,
            "idx": np.ascontiguousarray(idx_cols.T),
            "cnts": counts[g0:g0 + GPC].astype(np.float32).reshape(128, GPC // 128),
        })
    return in_maps


def _assemble(results):
    res = np.concatenate([r["out"][:RPC] for r in results], axis=0)
    res = res.reshape(NUM_GRAPHS, MAXA, NF)
    mask = np.concatenate([r["mask"] for r in results], axis=0) != 0
    return res, mask


def kernel(x, batch_ids, num_graphs, max_num_atoms):
    assert int(num_graphs) == NUM_GRAPHS and int(max_num_atoms) == MAXA
    from concourse.bass_utils import run_bass_kernel_spmd

    nc = _get_program()
    in_maps = _prep_inputs(x, batch_ids)
    res = run_bass_kernel_spmd(nc, in_maps, core_ids=list(range(NCORES)))
    return _assemble(res.results)
